# revision 4
# baseline (speedup 1.0000x reference)
"""3-layer GCN (GCNConv x3 + log_softmax) on 8 Trainium2 NeuronCores.

Strategy (dst-sharded graph parallel):
  - Nodes are partitioned into 8 contiguous ranges (12500 per core); core k
    owns dst range k and computes the output rows for its nodes.
  - Per layer: each core GEMMs its node slice H @ W (TensorE, W streamed
    against feature-major H^T tiles), writes the node-major product slice to
    DRAM, AllGathers the full [N,128] product, then aggregates:
    edge messages are fetched with dma_gather (rows land [128, cols, 128]
    edge-major), and the segment-sum over sorted-by-dst edges is done on the
    TensorEngine as one-hot matmuls: psum[feat, dst] += gathered^T-contract
    with S, where S[e, j] = (j == dstmod_e) * norm_e is built per 128-edge
    batch in ONE fused DVE tensor_scalar (is_equal then mult).
  - norm_e = dinv[src] * dinv[dst] folds the whole GCN normalization into S,
    so epilogues are a single ScalarE activation: relu(psum + bias) -> bf16
    H^T, which directly feeds the next layer's GEMM as the stationary operand.
  - Layer 3 flips matmul operands to get node-major psum[dst, feat] and runs
    log_softmax inline (exp with accumulated row-sum on ScalarE).

All feature data is bf16 (fp32 psum accumulation); indices int16 (gathers are
split over 4 source windows of 25000 rows to fit int16).
"""

import os
import sys

for _p in ("/opt/trn_rl_repo",):
    if os.path.isdir(_p) and _p not in sys.path:
        sys.path.insert(0, _p)

import numpy as np
import ml_dtypes

import concourse.bacc as bacc
import concourse.bass as bass
import concourse.tile as tile
from concourse import mybir, library_config
from concourse.bass_utils import run_bass_kernel_spmd
from concourse._compat import cdiv

BF16 = mybir.dt.bfloat16
F32 = mybir.dt.float32
I16 = mybir.dt.int16
NP_BF16 = ml_dtypes.bfloat16


# ----------------------------------------------------------------------------
# configuration
# ----------------------------------------------------------------------------
def full_cfg():
    return dict(N=100000, F=128, C=40, NCORES=8, BLK=128, SBB=8, NWIN=4)


def derive(cfg):
    d = dict(cfg)
    d["NPC"] = cfg["N"] // cfg["NCORES"]
    assert d["NPC"] * cfg["NCORES"] == cfg["N"]
    d["WIN"] = cdiv(cfg["N"], cfg["NWIN"])
    assert d["WIN"] <= 32767
    d["NBLK"] = cdiv(d["NPC"], cfg["BLK"])
    d["NSB"] = cdiv(d["NBLK"], cfg["SBB"])
    return d


# ----------------------------------------------------------------------------
# schedule: uniform (core-invariant) batch layout
# ----------------------------------------------------------------------------
class Sched:
    """Column layout for the padded edge-slot arrays.

    Global columns are ordered superblock-major; within a superblock,
    window-major; within a window, block-ascending. Each (blk, w) run owns
    nbatch[blk, w] columns of 128 slots.
    """

    def __init__(self, d, nbatch):
        NBLK, NWIN, SBB, NSB = d["NBLK"], d["NWIN"], d["SBB"], d["NSB"]
        self.nbatch = nbatch  # [NBLK, NWIN]
        self.sb_blocks = [
            list(range(sb * SBB, min((sb + 1) * SBB, NBLK))) for sb in range(NSB)
        ]
        self.sb_base = []
        self.w_off = []  # [sb][w] -> col offset within sb
        self.w_ncols = []  # [sb][w]
        self.colstart = np.zeros((NBLK, NWIN), dtype=np.int64)
        base = 0
        for sb in range(NSB):
            self.sb_base.append(base)
            offs, ncs = [], []
            off = 0
            for w in range(NWIN):
                offs.append(off)
                n = 0
                for b in self.sb_blocks[sb]:
                    self.colstart[b, w] = base + off + n
                    n += int(nbatch[b, w])
                ncs.append(n)
                off += n
            self.w_off.append(offs)
            self.w_ncols.append(ncs)
            base += off
        self.sb_ncols = [
            sum(self.w_ncols[sb]) for sb in range(NSB)
        ]
        self.totcols = base

    def block_cols(self, b):
        """All (global col, w) pairs of block b, in (w, batch) order."""
        out = []
        for w in range(self.nbatch.shape[1]):
            c0 = int(self.colstart[b, w])
            for j in range(int(self.nbatch[b, w])):
                out.append((c0 + j, w))
        return out


def prep_graph(d, edge_index):
    """Host-side graph prep. Returns (sched, per_core_arrays, dinv-etc)."""
    N, NPC, BLK, NWIN, WIN = d["N"], d["NPC"], d["BLK"], d["NWIN"], d["WIN"]
    NBLK, NCORES = d["NBLK"], d["NCORES"]

    src = np.asarray(edge_index[0], dtype=np.int64)
    dst = np.asarray(edge_index[1], dtype=np.int64)
    loop = np.arange(N, dtype=np.int64)
    src = np.concatenate([src, loop])
    dst = np.concatenate([dst, loop])

    deg = np.bincount(dst, minlength=N).astype(np.float64)
    dinv = (1.0 / np.sqrt(deg)).astype(np.float32)
    normv = (dinv[src] * dinv[dst]).astype(np.float32)

    core = dst // NPC
    rel = dst % NPC
    lblk = rel // BLK
    dmod = rel % BLK
    win = src // WIN

    # counts per (core, blk, win) -> global schedule
    key = (core * NBLK + lblk) * NWIN + win
    counts = np.bincount(key, minlength=NCORES * NBLK * NWIN).reshape(
        NCORES, NBLK, NWIN
    )
    nbatch = np.ceil(counts / 128).astype(np.int64).max(axis=0)  # [NBLK, NWIN]
    sched = Sched(d, nbatch)

    # rank of each edge within its (core, blk, win) group
    order = np.lexsort((win, lblk, core))
    k_sorted = key[order]
    grp_first = np.zeros(len(k_sorted), dtype=np.int64)
    newgrp = np.ones(len(k_sorted), dtype=bool)
    newgrp[1:] = k_sorted[1:] != k_sorted[:-1]
    first_pos = np.where(newgrp)[0]
    grp_id = np.cumsum(newgrp) - 1
    rank_sorted = np.arange(len(k_sorted)) - first_pos[grp_id]
    rank = np.empty(len(k_sorted), dtype=np.int64)
    rank[order] = rank_sorted

    # slot assignment (within the edge's own core's array)
    col = sched.colstart[lblk, win] + rank // 128
    part = rank % 128
    slot = col * 128 + part

    TOT = sched.totcols * 128
    per_core = []
    for c in range(NCORES):
        m = core == c
        lidx = np.zeros(TOT, dtype=np.int16)
        dm = np.full(TOT, -1.0, dtype=np.float32)
        nv = np.zeros(TOT, dtype=np.float32)
        s = slot[m]
        lidx[s] = (src[m] - win[m] * WIN).astype(np.int16)
        dm[s] = dmod[m].astype(np.float32)
        nv[s] = normv[m]
        # idx wrap: slot i -> [i % 16, i // 16]; replicate across 8 groups
        wrapped = lidx.reshape(-1, 16).T  # [16, TOT/16]
        idx128 = np.tile(wrapped, (8, 1))  # [128, TOT/16]
        # dstmod / norm: slot i -> [i % 128, i // 128]
        dm128 = dm.reshape(-1, 128).T  # [128, TOTCOLS]
        nv128 = nv.reshape(-1, 128).T
        per_core.append(
            dict(
                idx=np.ascontiguousarray(idx128),
                dstmod=np.ascontiguousarray(dm128),
                normv=np.ascontiguousarray(nv128),
            )
        )
    return sched, per_core


# ----------------------------------------------------------------------------
# kernel builder
# ----------------------------------------------------------------------------
def build(d, sched):
    N, F, C, NPC, BLK = d["N"], d["F"], d["C"], d["NPC"], d["BLK"]
    NBLK, NSB, NWIN, WIN, NCORES = d["NBLK"], d["NSB"], d["NWIN"], d["WIN"], d["NCORES"]
    TOTCOLS = sched.totcols
    MAXSBCOLS = max(sched.sb_ncols)

    nc = bacc.Bacc(
        "TRN2", target_bir_lowering=False, debug=False, num_devices=NCORES
    )

    xT = nc.dram_tensor("xT", [F, NPC], BF16, kind="ExternalInput")
    Ws = [
        nc.dram_tensor(f"W{i}", [F, F], BF16, kind="ExternalInput") for i in range(3)
    ]
    b1 = nc.dram_tensor("b1", [F, 1], F32, kind="ExternalInput")
    b2 = nc.dram_tensor("b2", [F, 1], F32, kind="ExternalInput")
    b3b = nc.dram_tensor("b3b", [128, C], F32, kind="ExternalInput")
    iota_in = nc.dram_tensor("iota", [128, 128], BF16, kind="ExternalInput")
    idx_in = nc.dram_tensor("idx", [128, TOTCOLS * 8], I16, kind="ExternalInput")
    dstmod_in = nc.dram_tensor("dstmod", [128, TOTCOLS], F32, kind="ExternalInput")
    normv_in = nc.dram_tensor("normv", [128, TOTCOLS], F32, kind="ExternalInput")
    out = nc.dram_tensor("out", [NPC, C], F32, kind="ExternalOutput")

    with tile.TileContext(nc) as tc:
        with (
            tc.tile_pool(name="const", bufs=1) as constp,
            tc.tile_pool(name="h", bufs=1) as hp,
            tc.tile_pool(name="gath", bufs=2) as gathp,
            tc.tile_pool(name="idxp", bufs=2) as idxp,
            tc.tile_pool(name="sp", bufs=8) as sp,
            tc.tile_pool(name="xw", bufs=4) as xwp,
            tc.tile_pool(name="ep", bufs=4) as epp,
            tc.tile_pool(name="ps_sb", bufs=2, space="PSUM") as ps_sb,
            tc.tile_pool(name="ps_blk", bufs=3, space="PSUM") as ps_blk,
            tc.tile_pool(name="dram", bufs=1, space="DRAM") as dramp,
        ):
            nc.gpsimd.load_library(library_config.mlp)

            # resident constants
            iota = constp.tile([128, 128], BF16, tag="iota")
            nc.sync.dma_start(iota[:], iota_in[:])
            wt = []
            for i in range(3):
                w = constp.tile([F, F], BF16, tag=f"w{i}")
                nc.sync.dma_start(w[:], Ws[i][:])
                wt.append(w)
            b1t = constp.tile([F, 1], F32, tag="b1")
            nc.sync.dma_start(b1t[:], b1[:])
            b2t = constp.tile([F, 1], F32, tag="b2")
            nc.sync.dma_start(b2t[:], b2[:])
            b3t = constp.tile([128, C], F32, tag="b3")
            nc.sync.dma_start(b3t[:], b3b[:])
            dmt = constp.tile([128, TOTCOLS], F32, tag="dm")
            nc.sync.dma_start(dmt[:], dstmod_in[:])
            nvt = constp.tile([128, TOTCOLS], F32, tag="nv")
            nc.sync.dma_start(nvt[:], normv_in[:])

            hA = hp.tile([F, NPC], BF16, tag="hA")
            hB = hp.tile([F, NPC], BF16, tag="hB")
            nc.sync.dma_start(hA[:], xT[:])

            # DRAM staging for the collective
            xw_slice = dramp.tile([NPC, F], BF16, tag="xw_slice")
            xw_full = dramp.tile([N, F], BF16, tag="xw_full")

            hcur = hA
            for L in range(3):
                # ---- GEMM: xw_slice[node, feat] = (H @ W_L) for own nodes
                for t in range(NBLK):
                    t0 = t * BLK
                    nt = min(BLK, NPC - t0)
                    ps = ps_blk.tile([128, F], F32, tag="gemm_ps")
                    nc.tensor.matmul(
                        ps[:nt, :],
                        hcur[:, t0 : t0 + nt],
                        wt[L][:],
                        start=True,
                        stop=True,
                    )
                    xw = xwp.tile([128, F], BF16, tag="xw")
                    nc.scalar.activation(
                        xw[:nt, :], ps[:nt, :], mybir.ActivationFunctionType.Copy
                    )
                    nc.sync.dma_start(xw_slice[t0 : t0 + nt, :], xw[:nt, :])

                # ---- AllGather the full product
                nc.gpsimd.collective_compute(
                    "AllGather",
                    mybir.AluOpType.bypass,
                    ins=[xw_slice.opt()],
                    outs=[xw_full.opt()],
                    replica_groups=[list(range(NCORES))],
                )

                # ---- aggregation over superblocks
                for sb in range(NSB):
                    blocks = sched.sb_blocks[sb]
                    base = sched.sb_base[sb]
                    ncols = sched.sb_ncols[sb]
                    idxt = idxp.tile([128, MAXSBCOLS * 8], I16, tag="idx")
                    nc.sync.dma_start(
                        idxt[:, : ncols * 8],
                        idx_in[:, base * 8 : (base + ncols) * 8],
                    )
                    g = gathp.tile([128, MAXSBCOLS, F], BF16, tag="g")
                    GCHUNK = 48  # cols per dma_gather (SWDGE ring limit)
                    for w in range(NWIN):
                        ncw_all = sched.w_ncols[sb][w]
                        for cc in range(0, ncw_all, GCHUNK):
                            c0 = sched.w_off[sb][w] + cc
                            ncw = min(GCHUNK, ncw_all - cc)
                            nc.gpsimd.dma_gather(
                                g[:, c0 : c0 + ncw, :],
                                xw_full[w * WIN : min((w + 1) * WIN, N), :],
                                idxt[:, c0 * 8 : (c0 + ncw) * 8],
                                ncw * 128,
                                ncw * 128,
                                F,
                                single_packet=False,
                            )
                    if L < 2:
                        pssb = ps_sb.tile([128, len(blocks) * BLK], F32, tag="pssb")
                    for bo, b in enumerate(blocks):
                        cols = sched.block_cols(b)
                        if L == 2:
                            psb = ps_blk.tile([128, F], F32, tag="gemm_ps")
                        for k, (gc, w) in enumerate(cols):
                            st = k == 0
                            sten = k == len(cols) - 1
                            lc = gc - base  # col within gathered tile
                            s = sp.tile([128, 128], BF16, tag="s")
                            nc.vector.tensor_scalar(
                                s[:],
                                iota[:],
                                dmt[:, gc : gc + 1],
                                nvt[:, gc : gc + 1],
                                mybir.AluOpType.is_equal,
                                mybir.AluOpType.mult,
                            )
                            if L < 2:
                                nc.tensor.matmul(
                                    pssb[:, bo * BLK : (bo + 1) * BLK],
                                    g[:, lc, :],
                                    s[:],
                                    start=st,
                                    stop=sten,
                                )
                            else:
                                nc.tensor.matmul(
                                    psb[:, :],
                                    s[:],
                                    g[:, lc, :],
                                    start=st,
                                    stop=sten,
                                )
                        t0 = b * BLK
                        nt = min(BLK, NPC - t0)
                        if L < 2:
                            hnext = hB if hcur is hA else hA
                            nc.scalar.activation(
                                hnext[:, t0 : t0 + nt],
                                pssb[:, bo * BLK : bo * BLK + nt],
                                mybir.ActivationFunctionType.Relu,
                                bias=(b1t if L == 0 else b2t)[:],
                            )
                        else:
                            # log_softmax epilogue, node-major psum [dst, feat]
                            t1 = epp.tile([128, C], F32, tag="t1")
                            nc.vector.tensor_tensor(
                                t1[:nt, :],
                                psb[:nt, :C],
                                b3t[:nt, :],
                                mybir.AluOpType.add,
                            )
                            e = epp.tile([128, C], F32, tag="e")
                            ss = epp.tile([128, 1], F32, tag="ss")
                            nc.scalar.activation(
                                e[:nt, :],
                                t1[:nt, :],
                                mybir.ActivationFunctionType.Exp,
                                accum_out=ss[:nt, :],
                            )
                            lns = epp.tile([128, 1], F32, tag="lns")
                            nc.scalar.activation(
                                lns[:nt, :],
                                ss[:nt, :],
                                mybir.ActivationFunctionType.Ln,
                            )
                            of = epp.tile([128, C], F32, tag="of")
                            nc.vector.tensor_scalar(
                                of[:nt, :],
                                t1[:nt, :],
                                lns[:nt, :],
                                None,
                                mybir.AluOpType.subtract,
                            )
                            nc.sync.dma_start(out[t0 : t0 + nt, :], of[:nt, :])
                if L < 2:
                    hcur = hB if hcur is hA else hA

    nc.compile()
    return nc


# ----------------------------------------------------------------------------
# host-side input prep
# ----------------------------------------------------------------------------
def make_in_maps(d, per_core, x, W1, b1, W2, b2, W3, b3):
    N, F, C, NPC, NCORES = d["N"], d["F"], d["C"], d["NPC"], d["NCORES"]
    x = np.asarray(x, dtype=np.float32)
    W3p = np.zeros((F, F), dtype=np.float32)
    W3p[:, : W3.shape[1]] = np.asarray(W3, dtype=np.float32)
    iota = np.broadcast_to(np.arange(128, dtype=np.float32), (128, 128))
    in_maps = []
    for c in range(NCORES):
        sl = slice(c * NPC, (c + 1) * NPC)
        in_maps.append(
            {
                "xT": np.ascontiguousarray(x[sl].T).astype(NP_BF16),
                "W0": np.asarray(W1, dtype=np.float32).astype(NP_BF16),
                "W1": np.asarray(W2, dtype=np.float32).astype(NP_BF16),
                "W2": W3p.astype(NP_BF16),
                "b1": np.asarray(b1, dtype=np.float32).reshape(F, 1),
                "b2": np.asarray(b2, dtype=np.float32).reshape(F, 1),
                "b3b": np.broadcast_to(
                    np.asarray(b3, dtype=np.float32), (128, C)
                ).copy(),
                "iota": iota.astype(NP_BF16),
                "idx": per_core[c]["idx"],
                "dstmod": per_core[c]["dstmod"],
                "normv": per_core[c]["normv"],
            }
        )
    return in_maps


_CACHE = {}


def run(d, edge_index, x, W1, b1, W2, b2, W3, b3, trace=False, trace_kwargs=None):
    key = "nc"
    if key not in _CACHE:
        sched, per_core = prep_graph(d, edge_index)
        nc = build(d, sched)
        _CACHE[key] = (nc, sched, per_core)
    nc, sched, per_core = _CACHE[key]
    in_maps = make_in_maps(d, per_core, x, W1, b1, W2, b2, W3, b3)
    res = run_bass_kernel_spmd(
        nc,
        in_maps,
        core_ids=list(range(d["NCORES"])),
        trace=trace,
        **(trace_kwargs or {}),
    )
    outs = [res.results[c]["out"] for c in range(d["NCORES"])]
    full = np.concatenate(outs, axis=0).astype(np.float32)
    return full, res


def kernel(x, edge_index, W1, b1, W2, b2, W3, b3):
    d = derive(full_cfg())
    out, _ = run(d, edge_index, x, W1, b1, W2, b2, W3, b3)
    return out


# revision 7
# speedup vs baseline: 1.7492x; 1.7492x over previous
"""3-layer GCN (GCNConv x3 + log_softmax) on 8 Trainium2 NeuronCores.

Strategy (dst-sharded graph parallel):
  - Nodes are partitioned into 8 contiguous ranges (12500 per core); core k
    owns dst range k and computes the output rows for its nodes.
  - Per layer: each core GEMMs its node slice H @ W (TensorE, W streamed
    against feature-major H^T tiles), writes the node-major product slice to
    DRAM, AllGathers the full [N,128] product, then aggregates:
    edge messages are fetched with dma_gather (rows land [128, cols, 128]
    edge-major), and the segment-sum over sorted-by-dst edges is done on the
    TensorEngine as one-hot matmuls: psum[feat, dst] += gathered^T-contract
    with S, where S[e, j] = (j == dstmod_e) * norm_e is built per 128-edge
    batch in ONE fused DVE tensor_scalar (is_equal then mult).
  - norm_e = dinv[src] * dinv[dst] folds the whole GCN normalization into S,
    so epilogues are a single ScalarE activation: relu(psum + bias) -> bf16
    H^T, which directly feeds the next layer's GEMM as the stationary operand.
  - Layer 3 flips matmul operands to get node-major psum[dst, feat] and runs
    log_softmax inline (exp with accumulated row-sum on ScalarE).

All feature data is bf16 (fp32 psum accumulation); indices int16 (gathers are
split over 4 source windows of 25000 rows to fit int16).
"""

import os
import sys

for _p in ("/opt/trn_rl_repo",):
    if os.path.isdir(_p) and _p not in sys.path:
        sys.path.insert(0, _p)

import numpy as np
import ml_dtypes

import concourse.bacc as bacc
import concourse.bass as bass
import concourse.tile as tile
from concourse import mybir, library_config
from concourse.bass_utils import run_bass_kernel_spmd
from concourse._compat import cdiv

BF16 = mybir.dt.bfloat16
F32 = mybir.dt.float32
I16 = mybir.dt.int16
NP_BF16 = ml_dtypes.bfloat16


# ----------------------------------------------------------------------------
# configuration
# ----------------------------------------------------------------------------
def full_cfg():
    return dict(N=100000, F=128, C=40, NCORES=8, BLK=128, SBB=8, NWIN=4)


def derive(cfg):
    d = dict(cfg)
    d["NPC"] = cfg["N"] // cfg["NCORES"]
    assert d["NPC"] * cfg["NCORES"] == cfg["N"]
    d["WIN"] = cdiv(cfg["N"], cfg["NWIN"])
    assert d["WIN"] <= 32767
    d["NBLK"] = cdiv(d["NPC"], cfg["BLK"])
    d["NSB"] = cdiv(d["NBLK"], cfg["SBB"])
    return d


# ----------------------------------------------------------------------------
# schedule: uniform (core-invariant) batch layout
# ----------------------------------------------------------------------------
class Sched:
    """Column layout for the padded edge-slot arrays.

    Global columns are ordered superblock-major; within a superblock,
    window-major; within a window, block-ascending. Each (blk, w) run owns
    nbatch[blk, w] columns of 128 slots.
    """

    def __init__(self, d, nbatch):
        NBLK, NWIN, SBB, NSB = d["NBLK"], d["NWIN"], d["SBB"], d["NSB"]
        self.nbatch = nbatch  # [NBLK, NWIN]
        self.sb_blocks = [
            list(range(sb * SBB, min((sb + 1) * SBB, NBLK))) for sb in range(NSB)
        ]
        self.sb_base = []
        self.w_off = []  # [sb][w] -> col offset within sb
        self.w_ncols = []  # [sb][w]
        self.colstart = np.zeros((NBLK, NWIN), dtype=np.int64)
        base = 0
        for sb in range(NSB):
            self.sb_base.append(base)
            offs, ncs = [], []
            off = 0
            for w in range(NWIN):
                offs.append(off)
                n = 0
                for b in self.sb_blocks[sb]:
                    self.colstart[b, w] = base + off + n
                    n += int(nbatch[b, w])
                ncs.append(n)
                off += n
            self.w_off.append(offs)
            self.w_ncols.append(ncs)
            base += off
        self.sb_ncols = [
            sum(self.w_ncols[sb]) for sb in range(NSB)
        ]
        self.totcols = base

    def block_cols(self, b):
        """All (global col, w) pairs of block b, in (w, batch) order."""
        out = []
        for w in range(self.nbatch.shape[1]):
            c0 = int(self.colstart[b, w])
            for j in range(int(self.nbatch[b, w])):
                out.append((c0 + j, w))
        return out


def prep_graph(d, edge_index):
    """Host-side graph prep. Returns (sched, per_core_arrays, dinv-etc)."""
    N, NPC, BLK, NWIN, WIN = d["N"], d["NPC"], d["BLK"], d["NWIN"], d["WIN"]
    NBLK, NCORES = d["NBLK"], d["NCORES"]

    src = np.asarray(edge_index[0], dtype=np.int64)
    dst = np.asarray(edge_index[1], dtype=np.int64)
    loop = np.arange(N, dtype=np.int64)
    src = np.concatenate([src, loop])
    dst = np.concatenate([dst, loop])

    deg = np.bincount(dst, minlength=N).astype(np.float64)
    dinv = (1.0 / np.sqrt(deg)).astype(np.float32)
    normv = (dinv[src] * dinv[dst]).astype(np.float32)

    core = dst // NPC
    rel = dst % NPC
    lblk = rel // BLK
    dmod = rel % BLK
    win = src // WIN

    # counts per (core, blk, win) -> global schedule
    key = (core * NBLK + lblk) * NWIN + win
    counts = np.bincount(key, minlength=NCORES * NBLK * NWIN).reshape(
        NCORES, NBLK, NWIN
    )
    nbatch = np.ceil(counts / 128).astype(np.int64).max(axis=0)  # [NBLK, NWIN]
    sched = Sched(d, nbatch)

    # rank of each edge within its (core, blk, win) group
    order = np.lexsort((win, lblk, core))
    k_sorted = key[order]
    grp_first = np.zeros(len(k_sorted), dtype=np.int64)
    newgrp = np.ones(len(k_sorted), dtype=bool)
    newgrp[1:] = k_sorted[1:] != k_sorted[:-1]
    first_pos = np.where(newgrp)[0]
    grp_id = np.cumsum(newgrp) - 1
    rank_sorted = np.arange(len(k_sorted)) - first_pos[grp_id]
    rank = np.empty(len(k_sorted), dtype=np.int64)
    rank[order] = rank_sorted

    # slot assignment (within the edge's own core's array)
    col = sched.colstart[lblk, win] + rank // 128
    part = rank % 128
    slot = col * 128 + part

    TOT = sched.totcols * 128
    per_core = []
    for c in range(NCORES):
        m = core == c
        lidx = np.zeros(TOT, dtype=np.int16)
        dm = np.full(TOT, -1.0, dtype=np.float32)
        nv = np.zeros(TOT, dtype=np.float32)
        s = slot[m]
        lidx[s] = (src[m] - win[m] * WIN).astype(np.int16)
        dm[s] = dmod[m].astype(np.float32)
        nv[s] = normv[m]
        # idx wrap: slot i -> [i % 16, i // 16]; replicate across 8 groups
        wrapped = lidx.reshape(-1, 16).T  # [16, TOT/16]
        idx128 = np.tile(wrapped, (8, 1))  # [128, TOT/16]
        # dstmod / norm: slot i -> [i % 128, i // 128]
        dm128 = dm.reshape(-1, 128).T  # [128, TOTCOLS]
        nv128 = nv.reshape(-1, 128).T
        per_core.append(
            dict(
                idx=np.ascontiguousarray(idx128),
                dstmod=np.ascontiguousarray(dm128),
                normv=np.ascontiguousarray(nv128),
            )
        )
    return sched, per_core


# ----------------------------------------------------------------------------
# kernel builder
# ----------------------------------------------------------------------------
def build(d, sched):
    N, F, C, NPC, BLK = d["N"], d["F"], d["C"], d["NPC"], d["BLK"]
    NBLK, NSB, NWIN, WIN, NCORES = d["NBLK"], d["NSB"], d["NWIN"], d["WIN"], d["NCORES"]
    TOTCOLS = sched.totcols
    MAXSBCOLS = max(sched.sb_ncols)

    nc = bacc.Bacc(
        "TRN2",
        target_bir_lowering=False,
        debug=False,
        num_devices=NCORES,
        num_swdge_queues=4,
    )

    xT = nc.dram_tensor("xT", [F, NPC], BF16, kind="ExternalInput")
    Ws = [
        nc.dram_tensor(f"W{i}", [F, F], BF16, kind="ExternalInput") for i in range(3)
    ]
    b1 = nc.dram_tensor("b1", [F, 1], F32, kind="ExternalInput")
    b2 = nc.dram_tensor("b2", [F, 1], F32, kind="ExternalInput")
    b3b = nc.dram_tensor("b3b", [128, C], F32, kind="ExternalInput")
    iota_in = nc.dram_tensor("iota", [128, 128], BF16, kind="ExternalInput")
    idx_in = nc.dram_tensor("idx", [128, TOTCOLS * 8], I16, kind="ExternalInput")
    dstmod_in = nc.dram_tensor("dstmod", [128, TOTCOLS], F32, kind="ExternalInput")
    normv_in = nc.dram_tensor("normv", [128, TOTCOLS], F32, kind="ExternalInput")
    out = nc.dram_tensor("out", [NPC, C], F32, kind="ExternalOutput")

    with tile.TileContext(nc) as tc:
        with (
            tc.tile_pool(name="const", bufs=1) as constp,
            tc.tile_pool(name="h", bufs=1) as hp,
            tc.tile_pool(name="gath", bufs=2) as gathp,
            tc.tile_pool(name="idxp", bufs=2) as idxp,
            tc.tile_pool(name="sp", bufs=8) as sp,
            tc.tile_pool(name="xw", bufs=4) as xwp,
            tc.tile_pool(name="ep", bufs=4) as epp,
            tc.tile_pool(name="ps_sb", bufs=2, space="PSUM") as ps_sb,
            tc.tile_pool(name="ps_blk", bufs=3, space="PSUM") as ps_blk,
            tc.tile_pool(name="dram", bufs=1, space="DRAM") as dramp,
        ):
            nc.gpsimd.load_library(library_config.mlp)

            # resident constants
            iota = constp.tile([128, 128], BF16, tag="iota")
            nc.sync.dma_start(iota[:], iota_in[:])
            wt = []
            for i in range(3):
                w = constp.tile([F, F], BF16, tag=f"w{i}")
                nc.sync.dma_start(w[:], Ws[i][:])
                wt.append(w)
            b1t = constp.tile([F, 1], F32, tag="b1")
            nc.sync.dma_start(b1t[:], b1[:])
            b2t = constp.tile([F, 1], F32, tag="b2")
            nc.sync.dma_start(b2t[:], b2[:])
            b3t = constp.tile([128, C], F32, tag="b3")
            nc.sync.dma_start(b3t[:], b3b[:])
            dmt = constp.tile([128, TOTCOLS], F32, tag="dm")
            nc.sync.dma_start(dmt[:], dstmod_in[:])
            nvt = constp.tile([128, TOTCOLS], F32, tag="nv")
            nc.sync.dma_start(nvt[:], normv_in[:])

            hA = hp.tile([F, NPC], BF16, tag="hA")
            hB = hp.tile([F, NPC], BF16, tag="hB")
            nc.sync.dma_start(hA[:], xT[:])

            # DRAM staging for the collective
            xw_slice = dramp.tile([NPC, F], BF16, tag="xw_slice")
            xw_full = dramp.tile([N, F], BF16, tag="xw_full")

            hcur = hA
            gq = [0]  # gather queue round-robin counter
            for L in range(3):
                # ---- GEMM: xw_slice[node, feat] = (H @ W_L) for own nodes
                for t in range(NBLK):
                    t0 = t * BLK
                    nt = min(BLK, NPC - t0)
                    ps = ps_blk.tile([128, F], F32, tag="gemm_ps")
                    nc.tensor.matmul(
                        ps[:nt, :],
                        hcur[:, t0 : t0 + nt],
                        wt[L][:],
                        start=True,
                        stop=True,
                    )
                    xw = xwp.tile([128, F], BF16, tag="xw")
                    nc.scalar.activation(
                        xw[:nt, :], ps[:nt, :], mybir.ActivationFunctionType.Copy
                    )
                    nc.sync.dma_start(xw_slice[t0 : t0 + nt, :], xw[:nt, :])

                # ---- AllGather the full product
                nc.gpsimd.collective_compute(
                    "AllGather",
                    mybir.AluOpType.bypass,
                    ins=[xw_slice.opt()],
                    outs=[xw_full.opt()],
                    replica_groups=[list(range(NCORES))],
                )

                # ---- aggregation over superblocks
                for sb in range(NSB):
                    blocks = sched.sb_blocks[sb]
                    base = sched.sb_base[sb]
                    ncols = sched.sb_ncols[sb]
                    idxt = idxp.tile([128, MAXSBCOLS * 8], I16, tag="idx")
                    nc.sync.dma_start(
                        idxt[:, : ncols * 8],
                        idx_in[:, base * 8 : (base + ncols) * 8],
                    )
                    g = gathp.tile([128, MAXSBCOLS, F], BF16, tag="g")
                    GCHUNK = 24  # cols per dma_gather (SWDGE ring limit)
                    for w in range(NWIN):
                        ncw_all = sched.w_ncols[sb][w]
                        for cc in range(0, ncw_all, GCHUNK):
                            c0 = sched.w_off[sb][w] + cc
                            ncw = min(GCHUNK, ncw_all - cc)
                            nc.gpsimd.dma_gather(
                                g[:, c0 : c0 + ncw, :],
                                xw_full[w * WIN : min((w + 1) * WIN, N), :],
                                idxt[:, c0 * 8 : (c0 + ncw) * 8],
                                ncw * 128,
                                ncw * 128,
                                F,
                                single_packet=False,
                                queue_num=gq[0] % 4,
                            )
                            gq[0] += 1
                    if L < 2:
                        pssb = ps_sb.tile([128, len(blocks) * BLK], F32, tag="pssb")
                    for bo, b in enumerate(blocks):
                        cols = sched.block_cols(b)
                        if L == 2:
                            psb = ps_blk.tile([128, F], F32, tag="gemm_ps")
                        for k, (gc, w) in enumerate(cols):
                            st = k == 0
                            sten = k == len(cols) - 1
                            lc = gc - base  # col within gathered tile
                            s = sp.tile([128, 128], BF16, tag="s")
                            nc.vector.tensor_scalar(
                                s[:],
                                iota[:],
                                dmt[:, gc : gc + 1],
                                nvt[:, gc : gc + 1],
                                mybir.AluOpType.is_equal,
                                mybir.AluOpType.mult,
                            )
                            if L < 2:
                                nc.tensor.matmul(
                                    pssb[:, bo * BLK : (bo + 1) * BLK],
                                    g[:, lc, :],
                                    s[:],
                                    start=st,
                                    stop=sten,
                                )
                            else:
                                nc.tensor.matmul(
                                    psb[:, :],
                                    s[:],
                                    g[:, lc, :],
                                    start=st,
                                    stop=sten,
                                )
                        t0 = b * BLK
                        nt = min(BLK, NPC - t0)
                        if L < 2:
                            hnext = hB if hcur is hA else hA
                            nc.scalar.activation(
                                hnext[:, t0 : t0 + nt],
                                pssb[:, bo * BLK : bo * BLK + nt],
                                mybir.ActivationFunctionType.Relu,
                                bias=(b1t if L == 0 else b2t)[:],
                            )
                        else:
                            # log_softmax epilogue, node-major psum [dst, feat]
                            t1 = epp.tile([128, C], F32, tag="t1")
                            nc.vector.tensor_tensor(
                                t1[:nt, :],
                                psb[:nt, :C],
                                b3t[:nt, :],
                                mybir.AluOpType.add,
                            )
                            e = epp.tile([128, C], F32, tag="e")
                            ss = epp.tile([128, 1], F32, tag="ss")
                            nc.scalar.activation(
                                e[:nt, :],
                                t1[:nt, :],
                                mybir.ActivationFunctionType.Exp,
                                accum_out=ss[:nt, :],
                            )
                            lns = epp.tile([128, 1], F32, tag="lns")
                            nc.scalar.activation(
                                lns[:nt, :],
                                ss[:nt, :],
                                mybir.ActivationFunctionType.Ln,
                            )
                            of = epp.tile([128, C], F32, tag="of")
                            nc.vector.tensor_scalar(
                                of[:nt, :],
                                t1[:nt, :],
                                lns[:nt, :],
                                None,
                                mybir.AluOpType.subtract,
                            )
                            nc.sync.dma_start(out[t0 : t0 + nt, :], of[:nt, :])
                if L < 2:
                    hcur = hB if hcur is hA else hA

    nc.compile()
    return nc


# ----------------------------------------------------------------------------
# host-side input prep
# ----------------------------------------------------------------------------
def make_in_maps(d, per_core, x, W1, b1, W2, b2, W3, b3):
    N, F, C, NPC, NCORES = d["N"], d["F"], d["C"], d["NPC"], d["NCORES"]
    x = np.asarray(x, dtype=np.float32)
    W3p = np.zeros((F, F), dtype=np.float32)
    W3p[:, : W3.shape[1]] = np.asarray(W3, dtype=np.float32)
    iota = np.broadcast_to(np.arange(128, dtype=np.float32), (128, 128))
    in_maps = []
    for c in range(NCORES):
        sl = slice(c * NPC, (c + 1) * NPC)
        in_maps.append(
            {
                "xT": np.ascontiguousarray(x[sl].T).astype(NP_BF16),
                "W0": np.asarray(W1, dtype=np.float32).astype(NP_BF16),
                "W1": np.asarray(W2, dtype=np.float32).astype(NP_BF16),
                "W2": W3p.astype(NP_BF16),
                "b1": np.asarray(b1, dtype=np.float32).reshape(F, 1),
                "b2": np.asarray(b2, dtype=np.float32).reshape(F, 1),
                "b3b": np.broadcast_to(
                    np.asarray(b3, dtype=np.float32), (128, C)
                ).copy(),
                "iota": iota.astype(NP_BF16),
                "idx": per_core[c]["idx"],
                "dstmod": per_core[c]["dstmod"],
                "normv": per_core[c]["normv"],
            }
        )
    return in_maps


_CACHE = {}


def run(d, edge_index, x, W1, b1, W2, b2, W3, b3, trace=False, trace_kwargs=None):
    key = "nc"
    if key not in _CACHE:
        sched, per_core = prep_graph(d, edge_index)
        nc = build(d, sched)
        _CACHE[key] = (nc, sched, per_core)
    nc, sched, per_core = _CACHE[key]
    in_maps = make_in_maps(d, per_core, x, W1, b1, W2, b2, W3, b3)
    res = run_bass_kernel_spmd(
        nc,
        in_maps,
        core_ids=list(range(d["NCORES"])),
        trace=trace,
        **(trace_kwargs or {}),
    )
    outs = [res.results[c]["out"] for c in range(d["NCORES"])]
    full = np.concatenate(outs, axis=0).astype(np.float32)
    return full, res


def kernel(x, edge_index, W1, b1, W2, b2, W3, b3):
    d = derive(full_cfg())
    out, _ = run(d, edge_index, x, W1, b1, W2, b2, W3, b3)
    return out
